# revision 10
# baseline (speedup 1.0000x reference)
"""Trainium2 Bass kernel for nn_Decoder (3-step LSTM decoder w/ Luong attention
+ conv1d entity heads). Data-parallel over batch: B=64 -> 8 cores x 8.

Decomposition (validated vs reference to 5e-7):
  - conv1d over feat=[enc, broadcast(o)] splits into a 3-tap matmul conv over
    enc (shared by both ent_heads calls) plus a per-batch bias vec@Kvec (with
    first/last-column variants for the SAME-padding edges).
  - attend(q) = tanh(mix @ Wa[:, :E].T + q @ Wa[:, E:].T + b) with
    mix = softmax(q.enc) @ enc.
All heavy matmuls run in bf16 (fp32 PSUM accumulation).
"""
import numpy as np
import ml_dtypes
from contextlib import ExitStack

import concourse.bass as bass
import concourse.bacc as bacc
import concourse.tile as tile
from concourse import mybir
from concourse.bass_utils import run_bass_kernel_spmd
from concourse.masks import make_identity

B, S, E, R = 64, 2048, 256, 50
NCORES = 8
BC = B // NCORES          # batch per core = 8
NCH = S // 512            # 4 s-chunks of 512
F32 = mybir.dt.float32
BF16 = mybir.dt.bfloat16
Relu = mybir.ActivationFunctionType.Relu
Tanh = mybir.ActivationFunctionType.Tanh
Exp = mybir.ActivationFunctionType.Exp
OC = [R, R + S, R + 2 * S, R + 3 * S]   # output col starts: e1a,e2a,e1b,e2b


def _emit(ctx, tc, nc, io):
    P = 128
    wp = ctx.enter_context(tc.tile_pool(name="wp", bufs=1))
    ep = ctx.enter_context(tc.tile_pool(name="ep", bufs=1))
    sp = ctx.enter_context(tc.tile_pool(name="sp", bufs=2))
    bigp = ctx.enter_context(tc.tile_pool(name="bigp", bufs=1))
    rp = ctx.enter_context(tc.tile_pool(name="rp", bufs=4))
    pg = ctx.enter_context(tc.tile_pool(name="pg", bufs=1, space="PSUM"))
    psc = ctx.enter_context(tc.tile_pool(name="psc", bufs=1, space="PSUM"))
    pcv = ctx.enter_context(tc.tile_pool(name="pcv", bufs=2, space="PSUM"))
    psm = ctx.enter_context(tc.tile_pool(name="psm", bufs=2, space="PSUM"))
    pent = ctx.enter_context(tc.tile_pool(name="pent", bufs=1, space="PSUM"))

    dma = nc.sync.dma_start

    # ---- weights / constants into SBUF ----
    def wload(name, shape, dt):
        t = wp.tile(shape, dt, name=name)
        dma(out=t[:], in_=io[name].ap())
        return t

    W_ihT = wload("W_ihT", [P, 2, 4 * E], BF16)
    W_hhT = wload("W_hhT", [P, 2, 4 * E], BF16)
    Wa_mT = wload("Wa_mT", [P, 2, E], BF16)
    Wa_qT = wload("Wa_qT", [P, 2, E], BF16)
    Kv_i = wload("Kv_i", [P, 2, E], BF16)
    Kv_f = wload("Kv_f", [P, 2, E], BF16)
    Kv_l = wload("Kv_l", [P, 2, E], BF16)
    Kenc = wload("Kenc", [P, 3, 2, 2, P], BF16)
    W_relT = wload("W_relT", [P, 2, R], BF16)
    Went = wload("Went", [P, 2, 2], BF16)
    bias_g = wload("bias_g", [1, 4 * E], BF16)
    b_attn = wload("b_attn", [1, E], BF16)
    b_conv = wload("b_conv", [1, E], BF16)
    b_rel = wload("b_rel", [1, R], BF16)
    bent = wload("bent", [2, 1], F32)
    xT = wload("xT", [P, 3, 2, BC], BF16)
    h0T = wload("h0T", [P, 2, BC], BF16)
    c0 = wload("c0", [BC, E], F32)

    ones_bf = wp.tile([1, BC], BF16, name="ones_bf")
    nc.vector.memset(ones_bf[:], 1.0)
    id_bf = wp.tile([P, P], BF16, name="id_bf")
    make_identity(nc, id_bf[:])
    id_f32 = wp.tile([P, P], F32, name="id_f32")
    make_identity(nc, id_f32[:])

    # ---- encoder tiles (both layouts, bf16, all 8 batches resident) ----
    encT = []   # [c(2x128 part), s] layout
    encS = []   # [s(16x128 part), c] layout
    for b in range(BC):
        tcs = bigp.tile([P, 2, S], BF16, name=f"encT{b}")
        for ch in range(2):
            dma(out=tcs[:, ch, :], in_=io["enc_cs"].ap()[b, ch * P:(ch + 1) * P, :])
        encT.append(tcs)
        tsc = bigp.tile([P, 16, E], BF16, name=f"encS{b}")
        dma(out=tsc[:], in_=io["enc_sc"].ap()[b].rearrange("(j p) c -> p j c", p=P))
        encS.append(tsc)

    out_ap = io["out"].ap()

    # ---- helper: transpose [BC, 2*P] sbuf -> [P, 2, BC] sbuf ----
    def transpose_to(src, dt, idt, name):
        dst = ep.tile([P, 2, BC], dt, name=name, bufs=2)
        for ch in range(2):
            pt = psm.tile([P, BC], dt, name="pt_tr", tag="ps")
            nc.tensor.transpose(pt[:], src[:, ch * P:(ch + 1) * P], idt[:BC, :BC])
            nc.scalar.copy(dst[:, ch, :], pt[:])
        return dst

    # ---- LSTM steps (batched over BC on partitions) ----
    def lstm_step(t, hT, c_prev):
        gps = pg.tile([BC, 4 * E], F32, name="gates")
        for nch in range(2):
            o_sl = gps[:, nch * 512:(nch + 1) * 512]
            first = True
            for kh in range(2):
                nc.tensor.matmul(o_sl, xT[:, t, kh, :], W_ihT[:, kh, nch * 512:(nch + 1) * 512],
                                 start=first, stop=False); first = False
                nc.tensor.matmul(o_sl, hT[:, kh, :], W_hhT[:, kh, nch * 512:(nch + 1) * 512],
                                 start=False, stop=False)
            nc.tensor.matmul(o_sl, ones_bf[:], bias_g[:, nch * 512:(nch + 1) * 512],
                             start=False, stop=True)
        # i,f,g,o slices; sigmoid via tanh: sig(x)=0.5*tanh(x/2)+0.5
        s_if = ep.tile([BC, 512], F32, name="s_if", bufs=1)
        nc.scalar.activation(s_if[:], gps[:, 0:512], Tanh, scale=0.5)
        nc.vector.tensor_scalar(s_if[:], s_if[:], 0.5, 0.5,
                                op0=mybir.AluOpType.mult, op1=mybir.AluOpType.add)
        t_g = ep.tile([BC, E], F32, name="t_g", bufs=1)
        nc.scalar.activation(t_g[:], gps[:, 512:768], Tanh)
        s_o = ep.tile([BC, E], F32, name="s_o", bufs=1)
        nc.scalar.activation(s_o[:], gps[:, 768:1024], Tanh, scale=0.5)
        nc.vector.tensor_scalar(s_o[:], s_o[:], 0.5, 0.5,
                                op0=mybir.AluOpType.mult, op1=mybir.AluOpType.add)
        c2 = ep.tile([BC, E], F32, name="c2", bufs=2)
        nc.vector.tensor_mul(c2[:], s_if[:, 256:512], c_prev[:])
        tmp = ep.tile([BC, E], F32, name="tmp_ig", bufs=1)
        nc.vector.tensor_mul(tmp[:], s_if[:, 0:256], t_g[:])
        nc.vector.tensor_add(c2[:], c2[:], tmp[:])
        tc2 = ep.tile([BC, E], F32, name="tc2", bufs=1)
        nc.scalar.activation(tc2[:], c2[:], Tanh)
        h2 = ep.tile([BC, E], BF16, name="h2", bufs=2)
        nc.vector.tensor_mul(h2[:], s_o[:], tc2[:])
        h2T = transpose_to(h2, BF16, id_bf, f"h2T_{t}")
        return h2, h2T, c2

    h1, h1T, c1 = lstm_step(0, h0T, c0)
    h2, h2T, c2 = lstm_step(1, h1T, c1)
    h3, h3T, c3 = lstm_step(2, h2T, c2)

    # ---- attention ----
    def attend(qT, tag):
        # masked qT: qTm[:, ch, b, :] has q_b in column b, zeros elsewhere, so
        # per-b matvecs accumulate into one [BC, 512] psum without row offsets
        qTm = sp.tile([P, 2, BC, BC], BF16, name="qTm", bufs=2)
        nc.vector.memset(qTm[:], 0.0)
        for ch in range(2):
            for b in range(BC):
                nc.vector.tensor_copy(qTm[:, ch, b, b:b + 1], qT[:, ch, b:b + 1])
        sc = sp.tile([BC, S], F32, name="sc", bufs=1)
        for j in range(NCH):
            sps = psc.tile([BC, 512], F32, name="sc_ps")
            for b in range(BC):
                for ch in range(2):
                    nc.tensor.matmul(sps[:], qTm[:, ch, b, :],
                                     encT[b][:, ch, j * 512:(j + 1) * 512],
                                     start=(b == 0 and ch == 0),
                                     stop=(b == BC - 1 and ch == 1))
            nc.vector.tensor_copy(sc[:, j * 512:(j + 1) * 512], sps[:])
        mx = ep.tile([BC, 1], F32, name="mx", bufs=2)
        nc.vector.reduce_max(mx[:], sc[:], axis=mybir.AxisListType.X)
        nc.vector.tensor_scalar_mul(mx[:], mx[:], -1.0)
        sm = ep.tile([BC, 1], F32, name="sm", bufs=2)
        nc.scalar.activation(sc[:], sc[:], Exp, bias=mx[:], accum_out=sm[:])
        rs = ep.tile([BC, 1], F32, name="rs", bufs=2)
        nc.vector.reciprocal(rs[:], sm[:])
        att = sp.tile([BC, S], BF16, name="att", bufs=1)
        nc.vector.tensor_scalar_mul(att[:], sc[:], rs[:])
        # transpose attn to [s,partition] tiles: [128,16,BC]
        attT = sp.tile([P, 16, BC], BF16, name="attT", bufs=2)
        for j in range(16):
            pt = psm.tile([P, BC], BF16, name="pt_at", tag="ps")
            nc.tensor.transpose(pt[:], att[:, j * P:(j + 1) * P], id_bf[:BC, :BC])
            nc.scalar.copy(attT[:, j, :], pt[:])
        # mix per b: accumulate over 16 s-tiles into [BC,E] psum (row b valid);
        # engines can't address partition offset b, so copy the full tile,
        # PE-transpose it, and pick column b (a free-dim offset).
        mixT = ep.tile([P, 2, BC], BF16, name=f"mixT_{tag}", bufs=2)
        for b in range(BC):
            mps = psm.tile([BC, E], F32, name="mix_ps", tag="ps")
            for j in range(16):
                nc.tensor.matmul(mps[:], attT[:, j, :], encS[b][:, j, :],
                                 start=(j == 0), stop=(j == 15))
            mfull = ep.tile([BC, E], BF16, name="mfull", bufs=2)
            nc.scalar.copy(mfull[:], mps[:])
            for ch in range(2):
                pt = psm.tile([P, BC], BF16, name="pt_mx", tag="ps")
                nc.tensor.transpose(pt[:], mfull[:, ch * P:(ch + 1) * P],
                                    id_bf[:BC, :BC])
                nc.vector.tensor_copy(mixT[:, ch, b:b + 1], pt[:, b:b + 1])
        aps = psm.tile([BC, E], F32, name="ao_ps", tag="ps")
        for ch in range(2):
            nc.tensor.matmul(aps[:], mixT[:, ch, :], Wa_mT[:, ch, :],
                             start=(ch == 0), stop=False)
        for ch in range(2):
            nc.tensor.matmul(aps[:], qT[:, ch, :], Wa_qT[:, ch, :],
                             start=False, stop=False)
        nc.tensor.matmul(aps[:], ones_bf[:], b_attn[:], start=False, stop=True)
        o = ep.tile([BC, E], BF16, name=f"out_{tag}", bufs=1)
        nc.scalar.activation(o[:], aps[:], Tanh)
        oT = transpose_to(o, BF16, id_bf, f"outT_{tag}")
        return o, oT

    out2, out2T = attend(h2T, "t2")
    out3, out3T = attend(h3T, "t3")
    out1, out1T = attend(h1T, "t1")

    # t1_out = out1 @ W_rel.T + b_rel -> out[:, 0:R]
    t1ps = psm.tile([BC, R], F32, name="t1_ps", tag="ps")
    for ch in range(2):
        nc.tensor.matmul(t1ps[:], out1T[:, ch, :], W_relT[:, ch, :],
                         start=(ch == 0), stop=False)
    nc.tensor.matmul(t1ps[:], ones_bf[:], b_rel[:], start=False, stop=True)
    t1sb = ep.tile([BC, R], F32, name="t1sb")
    nc.scalar.copy(t1sb[:], t1ps[:])
    dma(out=out_ap[:, 0:R], in_=t1sb[:])

    # ---- vbias variants: vb = o @ Kv_x + b_conv, transposed to [P,2,BC] ----
    def vbias(oT, Kv, tag):
        vps = psm.tile([BC, E], F32, name="vb_ps", tag="ps")
        for ch in range(2):
            nc.tensor.matmul(vps[:], oT[:, ch, :], Kv[:, ch, :],
                             start=(ch == 0), stop=False)
        nc.tensor.matmul(vps[:], ones_bf[:], b_conv[:], start=False, stop=True)
        vsb = ep.tile([BC, E], F32, name="vb_sb", bufs=2)
        nc.vector.tensor_copy(vsb[:], vps[:])
        return transpose_to(vsb, F32, id_f32, f"vbT_{tag}")

    vbA = [vbias(out2T, kv, f"a{i}") for i, kv in enumerate((Kv_i, Kv_f, Kv_l))]
    vbB = [vbias(out3T, kv, f"b{i}") for i, kv in enumerate((Kv_i, Kv_f, Kv_l))]

    # ---- conv + relu + entity-head reduction ----
    for b in range(BC):
        for j in range(NCH):
            s0 = j * 512
            cps = []
            for half in range(2):
                ps = pcv.tile([P, 512], F32, name="conv_ps")
                first = True
                # center tap (w=1) first: always full width, so the start=True
                # matmul initializes every psum element before partial taps add
                for w in (1, 0, 2):
                    lo = s0 + w - 1
                    ob, oe = 0, 512
                    if lo < 0:
                        ob, lo = 1, 0
                    elif lo + 512 > S:
                        oe = 511
                    for ch in range(2):
                        nc.tensor.matmul(ps[:, ob:oe], Kenc[:, w, ch, half, :],
                                         encT[b][:, ch, lo:lo + (oe - ob)],
                                         start=first, stop=(w == 2 and ch == 1))
                        first = False
                cps.append(ps)
            for v, vbs in enumerate((vbA, vbB)):
                ent_ps = pent.tile([2, 512], F32, name="ent_ps")
                for half in range(2):
                    r = rp.tile([P, 512], BF16, name="relu")
                    if v == 0:
                        nc.scalar.activation(r[:], cps[half][:], Relu,
                                             bias=vbs[0][:, half, b:b + 1])
                        if j == 0:
                            nc.scalar.activation(r[:, 0:1], cps[half][:, 0:1], Relu,
                                                 bias=vbs[1][:, half, b:b + 1])
                        if j == NCH - 1:
                            nc.scalar.activation(r[:, 511:512], cps[half][:, 511:512],
                                                 Relu, bias=vbs[2][:, half, b:b + 1])
                    else:
                        nc.vector.tensor_scalar(r[:], cps[half][:],
                                                vbs[0][:, half, b:b + 1], 0.0,
                                                op0=mybir.AluOpType.add,
                                                op1=mybir.AluOpType.max)
                        if j == 0:
                            nc.vector.tensor_scalar(r[:, 0:1], cps[half][:, 0:1],
                                                    vbs[1][:, half, b:b + 1], 0.0,
                                                    op0=mybir.AluOpType.add,
                                                    op1=mybir.AluOpType.max)
                        if j == NCH - 1:
                            nc.vector.tensor_scalar(r[:, 511:512], cps[half][:, 511:512],
                                                    vbs[2][:, half, b:b + 1], 0.0,
                                                    op0=mybir.AluOpType.add,
                                                    op1=mybir.AluOpType.max)
                    nc.tensor.matmul(ent_ps[:], Went[:, half, :], r[:],
                                     start=(half == 0), stop=(half == 1))
                esb = ep.tile([2, 512], F32, name="esb", bufs=2)
                nc.vector.tensor_scalar_add(esb[:], ent_ps[:], bent[:])
                dma(out=out_ap[b:b + 1, OC[2 * v] + s0:OC[2 * v] + s0 + 512],
                    in_=esb[0:1, :])
                dma(out=out_ap[b:b + 1, OC[2 * v + 1] + s0:OC[2 * v + 1] + s0 + 512],
                    in_=esb[1:2, :])


def build_nc():
    nc = bacc.Bacc("TRN2", target_bir_lowering=False, debug=False)
    io = {}

    def din(name, shape, dt):
        io[name] = nc.dram_tensor(name, shape, dt, kind="ExternalInput")

    din("enc_cs", [BC, E, S], BF16)
    din("enc_sc", [BC, S, E], BF16)
    din("W_ihT", [128, 2, 4 * E], BF16)
    din("W_hhT", [128, 2, 4 * E], BF16)
    din("Wa_mT", [128, 2, E], BF16)
    din("Wa_qT", [128, 2, E], BF16)
    din("Kv_i", [128, 2, E], BF16)
    din("Kv_f", [128, 2, E], BF16)
    din("Kv_l", [128, 2, E], BF16)
    din("Kenc", [128, 3, 2, 2, 128], BF16)
    din("W_relT", [128, 2, R], BF16)
    din("Went", [128, 2, 2], BF16)
    din("bias_g", [1, 4 * E], BF16)
    din("b_attn", [1, E], BF16)
    din("b_conv", [1, E], BF16)
    din("b_rel", [1, R], BF16)
    din("bent", [2, 1], F32)
    din("xT", [128, 3, 2, BC], BF16)
    din("h0T", [128, 2, BC], BF16)
    din("c0", [BC, E], F32)
    io["out"] = nc.dram_tensor("out", [BC, R + 4 * S], F32, kind="ExternalOutput")

    with ExitStack() as ctx:
        t = ctx.enter_context(tile.TileContext(nc))
        _emit(ctx, t, nc, io)
    nc.compile()
    return nc


def _pack2(w):  # [256, N] fp32 -> [128, 2, N]
    return np.ascontiguousarray(w.reshape(2, 128, -1).transpose(1, 0, 2))


def prepare_in_maps(inputs):
    bf = ml_dtypes.bfloat16
    enc = np.asarray(inputs["encoder_o"], np.float32)
    enc_bf = enc.astype(bf)
    enc_cs = np.ascontiguousarray(enc_bf.transpose(0, 2, 1))
    W_ih = np.asarray(inputs["W_ih"], np.float32)
    W_hh = np.asarray(inputs["W_hh"], np.float32)
    W_attn = np.asarray(inputs["W_attn"], np.float32)
    kern = np.asarray(inputs["W_conv"], np.float32).transpose(2, 1, 0)  # [3,2E,E]
    Kenc_ = kern[:, :E, :]
    Kv = kern[:, E:, :]
    Kv_i, Kv_f, Kv_l = Kv.sum(0), Kv[1] + Kv[2], Kv[0] + Kv[1]
    # Kenc pack [128, 3, 2, 2, 128]: [p,w,ch,half,m] = Kenc_[w, ch*128+p, half*128+m]
    kp = Kenc_.reshape(3, 2, 128, 2, 128).transpose(2, 0, 1, 3, 4)
    We = np.stack([np.asarray(inputs["W_ent1"])[0], np.asarray(inputs["W_ent2"])[0]], 1)
    x1 = np.broadcast_to(np.asarray(inputs["sos_emb"])[0], (B, E))
    x2 = np.asarray(inputs["rel_emb"])[np.asarray(inputs["r_in"]).astype(np.int64)]
    idx = np.arange(B)
    k1 = np.asarray(inputs["k1"])[:, 0].astype(np.int64)
    k2 = np.asarray(inputs["k2"])[:, 0].astype(np.int64)
    x3 = enc[idx, k1] + enc[idx, k2]
    X = np.stack([x1, x2, x3], 0).astype(np.float32)      # [3,B,E]
    h0 = np.asarray(inputs["h0"], np.float32)[0]
    c0 = np.asarray(inputs["c0"], np.float32)

    shared = {
        "W_ihT": _pack2(W_ih.T).astype(bf),
        "W_hhT": _pack2(W_hh.T).astype(bf),
        "Wa_mT": _pack2(W_attn[:, :E].T).astype(bf),
        "Wa_qT": _pack2(W_attn[:, E:].T).astype(bf),
        "Kv_i": _pack2(Kv_i).astype(bf),
        "Kv_f": _pack2(Kv_f).astype(bf),
        "Kv_l": _pack2(Kv_l).astype(bf),
        "Kenc": np.ascontiguousarray(kp).astype(bf),
        "W_relT": _pack2(np.asarray(inputs["W_rel"], np.float32).T).astype(bf),
        "Went": _pack2(We).astype(bf),
        "bias_g": (np.asarray(inputs["b_ih"], np.float32)
                   + np.asarray(inputs["b_hh"], np.float32))[None].astype(bf),
        "b_attn": np.asarray(inputs["b_attn"], np.float32)[None].astype(bf),
        "b_conv": np.asarray(inputs["b_conv"], np.float32)[None].astype(bf),
        "b_rel": np.asarray(inputs["b_rel"], np.float32)[None].astype(bf),
        "bent": np.array([[np.asarray(inputs["b_ent1"]).ravel()[0]],
                          [np.asarray(inputs["b_ent2"]).ravel()[0]]], np.float32),
    }
    in_maps = []
    for c in range(NCORES):
        sl = slice(c * BC, (c + 1) * BC)
        m = dict(shared)
        m["enc_cs"] = np.ascontiguousarray(enc_cs[sl])
        m["enc_sc"] = np.ascontiguousarray(enc_bf[sl])
        # xT pack [128, 3, 2, BC]
        xs = X[:, sl]                                      # [3,BC,E]
        m["xT"] = np.ascontiguousarray(
            xs.transpose(2, 0, 1).reshape(2, 128, 3, BC).transpose(1, 2, 0, 3)
        ).astype(bf)
        m["h0T"] = np.ascontiguousarray(
            h0[sl].T.reshape(2, 128, BC).transpose(1, 0, 2)).astype(bf)
        m["c0"] = np.ascontiguousarray(c0[0, sl]) if c0.ndim == 3 else \
            np.ascontiguousarray(c0[sl])
        in_maps.append(m)
    return in_maps


_NC_CACHE = {}


def get_nc():
    if "nc" not in _NC_CACHE:
        _NC_CACHE["nc"] = build_nc()
    return _NC_CACHE["nc"]


def kernel(**inputs) -> np.ndarray:
    nc = get_nc()
    in_maps = prepare_in_maps(inputs)
    res = run_bass_kernel_spmd(nc, in_maps, core_ids=list(range(NCORES)))
    return np.concatenate([r["out"] for r in res.results], 0).astype(np.float32)


if __name__ == "__main__":
    import jax
    import reference as refmod
    with jax.default_device(jax.devices("cpu")[0]):
        inputs = {k: np.asarray(v) for k, v in refmod.setup_inputs().items()}
        expected = np.asarray(refmod.reference(**inputs))
    actual = kernel(**inputs)
    err = np.abs(actual - expected)
    print("max abs err:", err.max(), "rel:", err.max() / np.abs(expected).max())


# revision 16
# speedup vs baseline: 409.7780x; 409.7780x over previous
"""Trainium2 Bass kernel for nn_Decoder (3-step LSTM decoder w/ Luong attention
+ conv1d entity heads). Data-parallel over batch: B=64 -> 8 cores x 8.

Decomposition (validated vs reference to 5e-7):
  - conv1d over feat=[enc, broadcast(o)] splits into a 3-tap matmul conv over
    enc (shared by both ent_heads calls) plus a per-batch bias vec@Kvec (with
    first/last-column variants for the SAME-padding edges).
  - attend(q) = tanh(mix @ Wa[:, :E].T + q @ Wa[:, E:].T + b) with
    mix = softmax(q.enc) @ enc.
All heavy matmuls run in bf16 (fp32 PSUM accumulation).
"""
import numpy as np
import ml_dtypes
from contextlib import ExitStack

import concourse.bass as bass
import concourse.bacc as bacc
import concourse.tile as tile
from concourse import mybir
from concourse.bass_utils import run_bass_kernel_spmd
from concourse.masks import make_identity

B, S, E, R = 64, 2048, 256, 50
NCORES = 8
BC = B // NCORES          # batch per core = 8
NCH = S // 512            # 4 s-chunks of 512
F32 = mybir.dt.float32
BF16 = mybir.dt.bfloat16
Relu = mybir.ActivationFunctionType.Relu
Tanh = mybir.ActivationFunctionType.Tanh
Exp = mybir.ActivationFunctionType.Exp
OC = [R, R + S, R + 2 * S, R + 3 * S]   # output col starts: e1a,e2a,e1b,e2b


def _emit(ctx, tc, nc, io):
    P = 128
    wp = ctx.enter_context(tc.tile_pool(name="wp", bufs=1))
    ep = ctx.enter_context(tc.tile_pool(name="ep", bufs=1))
    sp = ctx.enter_context(tc.tile_pool(name="sp", bufs=2))
    bigp = ctx.enter_context(tc.tile_pool(name="bigp", bufs=1))
    rp = ctx.enter_context(tc.tile_pool(name="rp", bufs=4))
    pg = ctx.enter_context(tc.tile_pool(name="pg", bufs=1, space="PSUM"))
    psc = ctx.enter_context(tc.tile_pool(name="psc", bufs=2, space="PSUM"))
    pcv = ctx.enter_context(tc.tile_pool(name="pcv", bufs=3, space="PSUM"))
    psm = ctx.enter_context(tc.tile_pool(name="psm", bufs=1, space="PSUM"))
    
    dma = nc.sync.dma_start

    # ---- weights / constants into SBUF ----
    def wload(name, shape, dt):
        t = wp.tile(shape, dt, name=name)
        dma(out=t[:], in_=io[name].ap())
        return t

    W_ihT = wload("W_ihT", [P, 2, 4 * E], BF16)
    W_hhT = wload("W_hhT", [P, 2, 4 * E], BF16)
    Wa_mT = wload("Wa_mT", [P, 2, E], BF16)
    Wa_qT = wload("Wa_qT", [P, 2, E], BF16)
    Kv_i = wload("Kv_i", [P, 2, E], BF16)
    Kv_f = wload("Kv_f", [P, 2, E], BF16)
    Kv_l = wload("Kv_l", [P, 2, E], BF16)
    Kenc = wload("Kenc", [P, 3, 2, 2, P], BF16)
    W_relT = wload("W_relT", [P, 2, R], BF16)
    Went = wload("Went", [P, 2, 2], BF16)
    bias_g = wload("bias_g", [1, 4 * E], BF16)
    b_attn = wload("b_attn", [1, E], BF16)
    b_conv = wload("b_conv", [1, E], BF16)
    b_rel = wload("b_rel", [1, R], BF16)
    bent = wload("bent", [2, 1], F32)
    xT = wload("xT", [P, 3, 2, BC], BF16)
    h0T = wload("h0T", [P, 2, BC], BF16)
    c0 = wload("c0", [BC, E], F32)

    ones_bf = wp.tile([1, BC], BF16, name="ones_bf")
    nc.vector.memset(ones_bf[:], 1.0)
    id_bf = wp.tile([P, P], BF16, name="id_bf")
    make_identity(nc, id_bf[:])
    id_f32 = wp.tile([P, P], F32, name="id_f32")
    make_identity(nc, id_f32[:])

    # ---- encoder tiles (both layouts, bf16, all 8 batches resident) ----
    encT = []   # [c(2x128 part), s] layout
    encS = []   # [s(16x128 part), c] layout
    for b in range(BC):
        tcs = bigp.tile([P, 2, S], BF16, name=f"encT{b}")
        for ch in range(2):
            dma(out=tcs[:, ch, :], in_=io["enc_cs"].ap()[b, ch * P:(ch + 1) * P, :])
        encT.append(tcs)
    for b in range(BC):
        tsc = bigp.tile([P, 16, E], BF16, name=f"encS{b}")
        dma(out=tsc[:], in_=io["enc_sc"].ap()[b].rearrange("(j p) c -> p j c", p=P))
        encS.append(tsc)

    out_ap = io["out"].ap()

    # ---- helper: transpose [BC, 2*P] sbuf -> [P, 2, BC] sbuf ----
    def transpose_to(src, dt, idt, name):
        dst = ep.tile([P, 2, BC], dt, name=name, bufs=2)
        for ch in range(2):
            pt = psm.tile([P, BC], dt, name="pt_tr", tag="ps")
            nc.tensor.transpose(pt[:], src[:, ch * P:(ch + 1) * P], idt[:BC, :BC])
            nc.scalar.copy(dst[:, ch, :], pt[:])
        return dst

    # ---- LSTM steps (batched over BC on partitions) ----
    def lstm_step(t, hT, c_prev):
        gps = pg.tile([BC, 4 * E], F32, name="gates")
        for nch in range(2):
            o_sl = gps[:, nch * 512:(nch + 1) * 512]
            first = True
            for kh in range(2):
                nc.tensor.matmul(o_sl, xT[:, t, kh, :], W_ihT[:, kh, nch * 512:(nch + 1) * 512],
                                 start=first, stop=False); first = False
                nc.tensor.matmul(o_sl, hT[:, kh, :], W_hhT[:, kh, nch * 512:(nch + 1) * 512],
                                 start=False, stop=False)
            nc.tensor.matmul(o_sl, ones_bf[:], bias_g[:, nch * 512:(nch + 1) * 512],
                             start=False, stop=True)
        # i,f,g,o slices; sigmoid via tanh: sig(x)=0.5*tanh(x/2)+0.5
        s_if = ep.tile([BC, 512], F32, name="s_if", bufs=1)
        nc.scalar.activation(s_if[:], gps[:, 0:512], Tanh, scale=0.5)
        nc.vector.tensor_scalar(s_if[:], s_if[:], 0.5, 0.5,
                                op0=mybir.AluOpType.mult, op1=mybir.AluOpType.add)
        t_g = ep.tile([BC, E], F32, name="t_g", bufs=1)
        nc.scalar.activation(t_g[:], gps[:, 512:768], Tanh)
        s_o = ep.tile([BC, E], F32, name="s_o", bufs=1)
        nc.scalar.activation(s_o[:], gps[:, 768:1024], Tanh, scale=0.5)
        nc.vector.tensor_scalar(s_o[:], s_o[:], 0.5, 0.5,
                                op0=mybir.AluOpType.mult, op1=mybir.AluOpType.add)
        c2 = ep.tile([BC, E], F32, name="c2", bufs=2)
        nc.vector.tensor_mul(c2[:], s_if[:, 256:512], c_prev[:])
        tmp = ep.tile([BC, E], F32, name="tmp_ig", bufs=1)
        nc.vector.tensor_mul(tmp[:], s_if[:, 0:256], t_g[:])
        nc.vector.tensor_add(c2[:], c2[:], tmp[:])
        tc2 = ep.tile([BC, E], F32, name="tc2", bufs=1)
        nc.scalar.activation(tc2[:], c2[:], Tanh)
        h2 = ep.tile([BC, E], BF16, name="h2", bufs=2)
        nc.vector.tensor_mul(h2[:], s_o[:], tc2[:])
        h2T = transpose_to(h2, BF16, id_bf, f"h2T_{t}")
        return h2, h2T, c2

    h1, h1T, c1 = lstm_step(0, h0T, c0)
    h2, h2T, c2 = lstm_step(1, h1T, c1)
    h3, h3T, c3 = lstm_step(2, h2T, c2)

    # ---- attention: all 3 attends (q = h1, h2, h3) batched in ONE pass ----
    # row index r = a*BC + b (a = attend/step, b = batch). One sweep over the
    # encoder serves all three queries: 3x less PE streaming than per-attend.
    NQ = 3 * BC  # 24
    qTs = [h1T, h2T, h3T]
    qTm = sp.tile([P, 2, BC, NQ], BF16, name="qTm", bufs=1)
    nc.vector.memset(qTm[:], 0.0)
    for ch in range(2):
        for a in range(3):
            for b in range(BC):
                nc.vector.tensor_copy(qTm[:, ch, b, a * BC + b:a * BC + b + 1],
                                      qTs[a][:, ch, b:b + 1])
    sc = sp.tile([NQ, S], F32, name="sc", bufs=1)
    for j in range(NCH):
        sps = psc.tile([NQ, 512], F32, name="sc_ps", tag="seb")
        for b in range(BC):
            for ch in range(2):
                nc.tensor.matmul(sps[:], qTm[:, ch, b, :],
                                 encT[b][:, ch, j * 512:(j + 1) * 512],
                                 start=(b == 0 and ch == 0),
                                 stop=(b == BC - 1 and ch == 1))
        nc.vector.tensor_copy(sc[:, j * 512:(j + 1) * 512], sps[:])
    mx = ep.tile([NQ, 1], F32, name="mx")
    nc.vector.reduce_max(mx[:], sc[:], axis=mybir.AxisListType.X)
    nc.vector.tensor_scalar_mul(mx[:], mx[:], -1.0)
    sm = ep.tile([NQ, 1], F32, name="sm")
    nc.scalar.activation(sc[:], sc[:], Exp, bias=mx[:], accum_out=sm[:])
    rs = ep.tile([NQ, 1], F32, name="rs")
    nc.vector.reciprocal(rs[:], sm[:])
    att = sp.tile([NQ, S], BF16, name="att", bufs=1)
    nc.vector.tensor_scalar_mul(att[:], sc[:], rs[:])
    # transpose attn to [s-partition] tiles: [128, 16, NQ]
    attT = sp.tile([P, 16, NQ], BF16, name="attT", bufs=1)
    for j in range(16):
        pt = psm.tile([P, NQ], BF16, name="pt_at", tag="ps")
        nc.tensor.transpose(pt[:], att[:, j * P:(j + 1) * P], id_bf[:NQ, :NQ])
        nc.scalar.copy(attT[:, j, :], pt[:])
    # mix: one [NQ, E] accumulation per b; rows {b, BC+b, 2*BC+b} are valid.
    # Engines can't address partition offsets, so copy the full tile,
    # PE-transpose it, and pick columns (free-dim offsets).
    mixTs = [ep.tile([P, 2, BC], BF16, name=f"mixT_t{a + 1}", bufs=2)
             for a in range(3)]
    for b in range(BC):
        mps = psm.tile([NQ, E], F32, name="mix_ps", tag="ps")
        for j in range(16):
            nc.tensor.matmul(mps[:], attT[:, j, :], encS[b][:, j, :],
                             start=(j == 0), stop=(j == 15))
        mfull = ep.tile([NQ, E], BF16, name="mfull", bufs=2)
        nc.scalar.copy(mfull[:], mps[:])
        for ch in range(2):
            pt = psm.tile([P, NQ], BF16, name="pt_mx", tag="ps")
            nc.tensor.transpose(pt[:], mfull[:, ch * P:(ch + 1) * P],
                                id_bf[:NQ, :NQ])
            for a in range(3):
                nc.vector.tensor_copy(mixTs[a][:, ch, b:b + 1],
                                      pt[:, a * BC + b:a * BC + b + 1])

    def attend_out(mixT, qT, tag):
        aps = psm.tile([BC, E], F32, name="ao_ps", tag="ps")
        for ch in range(2):
            nc.tensor.matmul(aps[:], mixT[:, ch, :], Wa_mT[:, ch, :],
                             start=(ch == 0), stop=False)
        for ch in range(2):
            nc.tensor.matmul(aps[:], qT[:, ch, :], Wa_qT[:, ch, :],
                             start=False, stop=False)
        nc.tensor.matmul(aps[:], ones_bf[:], b_attn[:], start=False, stop=True)
        o = ep.tile([BC, E], BF16, name=f"out_{tag}", bufs=1)
        nc.scalar.activation(o[:], aps[:], Tanh)
        oT = transpose_to(o, BF16, id_bf, f"outT_{tag}")
        return o, oT

    out2, out2T = attend_out(mixTs[1], h2T, "t2")
    out3, out3T = attend_out(mixTs[2], h3T, "t3")
    out1, out1T = attend_out(mixTs[0], h1T, "t1")

    # t1_out = out1 @ W_rel.T + b_rel -> out[:, 0:R]
    t1ps = psm.tile([BC, R], F32, name="t1_ps", tag="ps")
    for ch in range(2):
        nc.tensor.matmul(t1ps[:], out1T[:, ch, :], W_relT[:, ch, :],
                         start=(ch == 0), stop=False)
    nc.tensor.matmul(t1ps[:], ones_bf[:], b_rel[:], start=False, stop=True)
    t1sb = ep.tile([BC, R], F32, name="t1sb")
    nc.scalar.copy(t1sb[:], t1ps[:])
    dma(out=out_ap[:, 0:R], in_=t1sb[:])

    # ---- vbias variants: vb = o @ Kv_x + b_conv, transposed to [P,2,BC] ----
    def vbias(oT, Kv, tag):
        vps = psm.tile([BC, E], F32, name="vb_ps", tag="ps")
        for ch in range(2):
            nc.tensor.matmul(vps[:], oT[:, ch, :], Kv[:, ch, :],
                             start=(ch == 0), stop=False)
        nc.tensor.matmul(vps[:], ones_bf[:], b_conv[:], start=False, stop=True)
        vsb = ep.tile([BC, E], F32, name="vb_sb", bufs=2)
        nc.vector.tensor_copy(vsb[:], vps[:])
        return transpose_to(vsb, F32, id_f32, f"vbT_{tag}")

    vbA = [vbias(out2T, kv, f"a{i}") for i, kv in enumerate((Kv_i, Kv_f, Kv_l))]
    vbB = [vbias(out3T, kv, f"b{i}") for i, kv in enumerate((Kv_i, Kv_f, Kv_l))]

    # ---- conv + relu + entity-head reduction ----
    for b in range(BC):
        for j in range(NCH):
            s0 = j * 512
            cps = []
            for half in range(2):
                ps = pcv.tile([P, 512], F32, name="conv_ps")
                first = True
                # center tap (w=1) first: always full width, so the start=True
                # matmul initializes every psum element before partial taps add
                for w in (1, 0, 2):
                    lo = s0 + w - 1
                    ob, oe = 0, 512
                    if lo < 0:
                        ob, lo = 1, 0
                    elif lo + 512 > S:
                        oe = 511
                    for ch in range(2):
                        nc.tensor.matmul(ps[:, ob:oe], Kenc[:, w, ch, half, :],
                                         encT[b][:, ch, lo:lo + (oe - ob)],
                                         start=first, stop=(w == 2 and ch == 1))
                        first = False
                cps.append(ps)
            for v, vbs in enumerate((vbA, vbB)):
                ent_ps = psc.tile([2, 512], F32, name="ent_ps", tag="seb")
                for half in range(2):
                    r = rp.tile([P, 512], BF16, name="relu")
                    if v == 0:
                        nc.scalar.activation(r[:], cps[half][:], Relu,
                                             bias=vbs[0][:, half, b:b + 1])
                        if j == 0:
                            nc.scalar.activation(r[:, 0:1], cps[half][:, 0:1], Relu,
                                                 bias=vbs[1][:, half, b:b + 1])
                        if j == NCH - 1:
                            nc.scalar.activation(r[:, 511:512], cps[half][:, 511:512],
                                                 Relu, bias=vbs[2][:, half, b:b + 1])
                    else:
                        nc.vector.tensor_scalar(r[:], cps[half][:],
                                                vbs[0][:, half, b:b + 1], 0.0,
                                                op0=mybir.AluOpType.add,
                                                op1=mybir.AluOpType.max)
                        if j == 0:
                            nc.vector.tensor_scalar(r[:, 0:1], cps[half][:, 0:1],
                                                    vbs[1][:, half, b:b + 1], 0.0,
                                                    op0=mybir.AluOpType.add,
                                                    op1=mybir.AluOpType.max)
                        if j == NCH - 1:
                            nc.vector.tensor_scalar(r[:, 511:512], cps[half][:, 511:512],
                                                    vbs[2][:, half, b:b + 1], 0.0,
                                                    op0=mybir.AluOpType.add,
                                                    op1=mybir.AluOpType.max)
                    nc.tensor.matmul(ent_ps[:], Went[:, half, :], r[:],
                                     start=(half == 0), stop=(half == 1))
                esb = ep.tile([2, 512], F32, name="esb", bufs=4)
                if (b + j + v) % 2 == 0:
                    nc.vector.tensor_scalar_add(esb[:], ent_ps[:], bent[:])
                else:
                    nc.scalar.activation(esb[:], ent_ps[:],
                                         mybir.ActivationFunctionType.Identity,
                                         bias=bent[:])
                dma(out=out_ap[b:b + 1, OC[2 * v] + s0:OC[2 * v] + s0 + 512],
                    in_=esb[0:1, :])
                dma(out=out_ap[b:b + 1, OC[2 * v + 1] + s0:OC[2 * v + 1] + s0 + 512],
                    in_=esb[1:2, :])


def build_nc():
    nc = bacc.Bacc("TRN2", target_bir_lowering=False, debug=False)
    io = {}

    def din(name, shape, dt):
        io[name] = nc.dram_tensor(name, shape, dt, kind="ExternalInput")

    din("enc_cs", [BC, E, S], BF16)
    din("enc_sc", [BC, S, E], BF16)
    din("W_ihT", [128, 2, 4 * E], BF16)
    din("W_hhT", [128, 2, 4 * E], BF16)
    din("Wa_mT", [128, 2, E], BF16)
    din("Wa_qT", [128, 2, E], BF16)
    din("Kv_i", [128, 2, E], BF16)
    din("Kv_f", [128, 2, E], BF16)
    din("Kv_l", [128, 2, E], BF16)
    din("Kenc", [128, 3, 2, 2, 128], BF16)
    din("W_relT", [128, 2, R], BF16)
    din("Went", [128, 2, 2], BF16)
    din("bias_g", [1, 4 * E], BF16)
    din("b_attn", [1, E], BF16)
    din("b_conv", [1, E], BF16)
    din("b_rel", [1, R], BF16)
    din("bent", [2, 1], F32)
    din("xT", [128, 3, 2, BC], BF16)
    din("h0T", [128, 2, BC], BF16)
    din("c0", [BC, E], F32)
    io["out"] = nc.dram_tensor("out", [BC, R + 4 * S], F32, kind="ExternalOutput")

    with ExitStack() as ctx:
        t = ctx.enter_context(tile.TileContext(nc))
        _emit(ctx, t, nc, io)
    nc.compile()
    return nc


def _pack2(w):  # [256, N] fp32 -> [128, 2, N]
    return np.ascontiguousarray(w.reshape(2, 128, -1).transpose(1, 0, 2))


def prepare_in_maps(inputs):
    bf = ml_dtypes.bfloat16
    enc = np.asarray(inputs["encoder_o"], np.float32)
    enc_bf = enc.astype(bf)
    enc_cs = np.ascontiguousarray(enc_bf.transpose(0, 2, 1))
    W_ih = np.asarray(inputs["W_ih"], np.float32)
    W_hh = np.asarray(inputs["W_hh"], np.float32)
    W_attn = np.asarray(inputs["W_attn"], np.float32)
    kern = np.asarray(inputs["W_conv"], np.float32).transpose(2, 1, 0)  # [3,2E,E]
    Kenc_ = kern[:, :E, :]
    Kv = kern[:, E:, :]
    Kv_i, Kv_f, Kv_l = Kv.sum(0), Kv[1] + Kv[2], Kv[0] + Kv[1]
    # Kenc pack [128, 3, 2, 2, 128]: [p,w,ch,half,m] = Kenc_[w, ch*128+p, half*128+m]
    kp = Kenc_.reshape(3, 2, 128, 2, 128).transpose(2, 0, 1, 3, 4)
    We = np.stack([np.asarray(inputs["W_ent1"])[0], np.asarray(inputs["W_ent2"])[0]], 1)
    x1 = np.broadcast_to(np.asarray(inputs["sos_emb"])[0], (B, E))
    x2 = np.asarray(inputs["rel_emb"])[np.asarray(inputs["r_in"]).astype(np.int64)]
    idx = np.arange(B)
    k1 = np.asarray(inputs["k1"])[:, 0].astype(np.int64)
    k2 = np.asarray(inputs["k2"])[:, 0].astype(np.int64)
    x3 = enc[idx, k1] + enc[idx, k2]
    X = np.stack([x1, x2, x3], 0).astype(np.float32)      # [3,B,E]
    h0 = np.asarray(inputs["h0"], np.float32)[0]
    c0 = np.asarray(inputs["c0"], np.float32)

    shared = {
        "W_ihT": _pack2(W_ih.T).astype(bf),
        "W_hhT": _pack2(W_hh.T).astype(bf),
        "Wa_mT": _pack2(W_attn[:, :E].T).astype(bf),
        "Wa_qT": _pack2(W_attn[:, E:].T).astype(bf),
        "Kv_i": _pack2(Kv_i).astype(bf),
        "Kv_f": _pack2(Kv_f).astype(bf),
        "Kv_l": _pack2(Kv_l).astype(bf),
        "Kenc": np.ascontiguousarray(kp).astype(bf),
        "W_relT": _pack2(np.asarray(inputs["W_rel"], np.float32).T).astype(bf),
        "Went": _pack2(We).astype(bf),
        "bias_g": (np.asarray(inputs["b_ih"], np.float32)
                   + np.asarray(inputs["b_hh"], np.float32))[None].astype(bf),
        "b_attn": np.asarray(inputs["b_attn"], np.float32)[None].astype(bf),
        "b_conv": np.asarray(inputs["b_conv"], np.float32)[None].astype(bf),
        "b_rel": np.asarray(inputs["b_rel"], np.float32)[None].astype(bf),
        "bent": np.array([[np.asarray(inputs["b_ent1"]).ravel()[0]],
                          [np.asarray(inputs["b_ent2"]).ravel()[0]]], np.float32),
    }
    in_maps = []
    for c in range(NCORES):
        sl = slice(c * BC, (c + 1) * BC)
        m = dict(shared)
        m["enc_cs"] = np.ascontiguousarray(enc_cs[sl])
        m["enc_sc"] = np.ascontiguousarray(enc_bf[sl])
        # xT pack [128, 3, 2, BC]
        xs = X[:, sl]                                      # [3,BC,E]
        m["xT"] = np.ascontiguousarray(
            xs.transpose(2, 0, 1).reshape(2, 128, 3, BC).transpose(1, 2, 0, 3)
        ).astype(bf)
        m["h0T"] = np.ascontiguousarray(
            h0[sl].T.reshape(2, 128, BC).transpose(1, 0, 2)).astype(bf)
        m["c0"] = np.ascontiguousarray(c0[0, sl]) if c0.ndim == 3 else \
            np.ascontiguousarray(c0[sl])
        in_maps.append(m)
    return in_maps


_NC_CACHE = {}


def get_nc():
    if "nc" not in _NC_CACHE:
        _NC_CACHE["nc"] = build_nc()
    return _NC_CACHE["nc"]


def kernel(**inputs) -> np.ndarray:
    nc = get_nc()
    in_maps = prepare_in_maps(inputs)
    res = run_bass_kernel_spmd(nc, in_maps, core_ids=list(range(NCORES)))
    return np.concatenate([r["out"] for r in res.results], 0).astype(np.float32)


if __name__ == "__main__":
    import jax
    import reference as refmod
    with jax.default_device(jax.devices("cpu")[0]):
        inputs = {k: np.asarray(v) for k, v in refmod.setup_inputs().items()}
        expected = np.asarray(refmod.reference(**inputs))
    actual = kernel(**inputs)
    err = np.abs(actual - expected)
    print("max abs err:", err.max(), "rel:", err.max() / np.abs(expected).max())


# revision 17
# speedup vs baseline: 411.9991x; 1.0054x over previous
"""Trainium2 Bass kernel for nn_Decoder (3-step LSTM decoder w/ Luong attention
+ conv1d entity heads). Data-parallel over batch: B=64 -> 8 cores x 8.

Decomposition (validated vs reference to 5e-7):
  - conv1d over feat=[enc, broadcast(o)] splits into a 3-tap matmul conv over
    enc (shared by both ent_heads calls) plus a per-batch bias vec@Kvec (with
    first/last-column variants for the SAME-padding edges).
  - attend(q) = tanh(mix @ Wa[:, :E].T + q @ Wa[:, E:].T + b) with
    mix = softmax(q.enc) @ enc.
All heavy matmuls run in bf16 (fp32 PSUM accumulation).
"""
import numpy as np
import ml_dtypes
from contextlib import ExitStack

import concourse.bass as bass
import concourse.bacc as bacc
import concourse.tile as tile
from concourse import mybir
from concourse.bass_utils import run_bass_kernel_spmd
from concourse.masks import make_identity

B, S, E, R = 64, 2048, 256, 50
NCORES = 8
BC = B // NCORES          # batch per core = 8
NCH = S // 512            # 4 s-chunks of 512
F32 = mybir.dt.float32
BF16 = mybir.dt.bfloat16
Relu = mybir.ActivationFunctionType.Relu
Tanh = mybir.ActivationFunctionType.Tanh
Exp = mybir.ActivationFunctionType.Exp
OC = [R, R + S, R + 2 * S, R + 3 * S]   # output col starts: e1a,e2a,e1b,e2b


def _emit(ctx, tc, nc, io):
    P = 128
    wp = ctx.enter_context(tc.tile_pool(name="wp", bufs=1))
    ep = ctx.enter_context(tc.tile_pool(name="ep", bufs=1))
    sp = ctx.enter_context(tc.tile_pool(name="sp", bufs=2))
    bigp = ctx.enter_context(tc.tile_pool(name="bigp", bufs=1))
    rp = ctx.enter_context(tc.tile_pool(name="rp", bufs=6))
    pg = ctx.enter_context(tc.tile_pool(name="pg", bufs=1, space="PSUM"))
    psc = ctx.enter_context(tc.tile_pool(name="psc", bufs=2, space="PSUM"))
    pcv = ctx.enter_context(tc.tile_pool(name="pcv", bufs=3, space="PSUM"))
    psm = ctx.enter_context(tc.tile_pool(name="psm", bufs=1, space="PSUM"))
    
    dma = nc.sync.dma_start

    # ---- weights / constants into SBUF ----
    def wload(name, shape, dt):
        t = wp.tile(shape, dt, name=name)
        dma(out=t[:], in_=io[name].ap())
        return t

    W_ihT = wload("W_ihT", [P, 2, 4 * E], BF16)
    W_hhT = wload("W_hhT", [P, 2, 4 * E], BF16)
    Wa_mT = wload("Wa_mT", [P, 2, E], BF16)
    Wa_qT = wload("Wa_qT", [P, 2, E], BF16)
    Kv_i = wload("Kv_i", [P, 2, E], BF16)
    Kv_f = wload("Kv_f", [P, 2, E], BF16)
    Kv_l = wload("Kv_l", [P, 2, E], BF16)
    Kenc = wload("Kenc", [P, 3, 2, 2, P], BF16)
    W_relT = wload("W_relT", [P, 2, R], BF16)
    Went = wload("Went", [P, 2, 2], BF16)
    bias_g = wload("bias_g", [1, 4 * E], BF16)
    b_attn = wload("b_attn", [1, E], BF16)
    b_conv = wload("b_conv", [1, E], BF16)
    b_rel = wload("b_rel", [1, R], BF16)
    bent = wload("bent", [2, 1], F32)
    xT = wload("xT", [P, 3, 2, BC], BF16)
    h0T = wload("h0T", [P, 2, BC], BF16)
    c0 = wload("c0", [BC, E], F32)

    ones_bf = wp.tile([1, BC], BF16, name="ones_bf")
    nc.vector.memset(ones_bf[:], 1.0)
    id_bf = wp.tile([P, P], BF16, name="id_bf")
    make_identity(nc, id_bf[:])
    id_f32 = wp.tile([P, P], F32, name="id_f32")
    make_identity(nc, id_f32[:])

    # ---- encoder tiles (both layouts, bf16, all 8 batches resident) ----
    encT = []   # [c(2x128 part), s] layout
    encS = []   # [s(16x128 part), c] layout
    for b in range(BC):
        tcs = bigp.tile([P, 2, S], BF16, name=f"encT{b}")
        for ch in range(2):
            dma(out=tcs[:, ch, :], in_=io["enc_cs"].ap()[b, ch * P:(ch + 1) * P, :])
        encT.append(tcs)
    for b in range(BC):
        tsc = bigp.tile([P, 16, E], BF16, name=f"encS{b}")
        dma(out=tsc[:], in_=io["enc_sc"].ap()[b].rearrange("(j p) c -> p j c", p=P))
        encS.append(tsc)

    out_ap = io["out"].ap()

    # ---- helper: transpose [BC, 2*P] sbuf -> [P, 2, BC] sbuf ----
    def transpose_to(src, dt, idt, name):
        dst = ep.tile([P, 2, BC], dt, name=name, bufs=2)
        for ch in range(2):
            pt = psm.tile([P, BC], dt, name="pt_tr", tag="ps")
            nc.tensor.transpose(pt[:], src[:, ch * P:(ch + 1) * P], idt[:BC, :BC])
            nc.scalar.copy(dst[:, ch, :], pt[:])
        return dst

    # ---- LSTM steps (batched over BC on partitions) ----
    def lstm_step(t, hT, c_prev):
        gps = pg.tile([BC, 4 * E], F32, name="gates")
        for nch in range(2):
            o_sl = gps[:, nch * 512:(nch + 1) * 512]
            first = True
            for kh in range(2):
                nc.tensor.matmul(o_sl, xT[:, t, kh, :], W_ihT[:, kh, nch * 512:(nch + 1) * 512],
                                 start=first, stop=False); first = False
                nc.tensor.matmul(o_sl, hT[:, kh, :], W_hhT[:, kh, nch * 512:(nch + 1) * 512],
                                 start=False, stop=False)
            nc.tensor.matmul(o_sl, ones_bf[:], bias_g[:, nch * 512:(nch + 1) * 512],
                             start=False, stop=True)
        # i,f,g,o slices; sigmoid via tanh: sig(x)=0.5*tanh(x/2)+0.5
        s_if = ep.tile([BC, 512], F32, name="s_if", bufs=1)
        nc.scalar.activation(s_if[:], gps[:, 0:512], Tanh, scale=0.5)
        nc.vector.tensor_scalar(s_if[:], s_if[:], 0.5, 0.5,
                                op0=mybir.AluOpType.mult, op1=mybir.AluOpType.add)
        t_g = ep.tile([BC, E], F32, name="t_g", bufs=1)
        nc.scalar.activation(t_g[:], gps[:, 512:768], Tanh)
        s_o = ep.tile([BC, E], F32, name="s_o", bufs=1)
        nc.scalar.activation(s_o[:], gps[:, 768:1024], Tanh, scale=0.5)
        nc.vector.tensor_scalar(s_o[:], s_o[:], 0.5, 0.5,
                                op0=mybir.AluOpType.mult, op1=mybir.AluOpType.add)
        c2 = ep.tile([BC, E], F32, name="c2", bufs=2)
        nc.vector.tensor_mul(c2[:], s_if[:, 256:512], c_prev[:])
        tmp = ep.tile([BC, E], F32, name="tmp_ig", bufs=1)
        nc.vector.tensor_mul(tmp[:], s_if[:, 0:256], t_g[:])
        nc.vector.tensor_add(c2[:], c2[:], tmp[:])
        tc2 = ep.tile([BC, E], F32, name="tc2", bufs=1)
        nc.scalar.activation(tc2[:], c2[:], Tanh)
        h2 = ep.tile([BC, E], BF16, name="h2", bufs=2)
        nc.vector.tensor_mul(h2[:], s_o[:], tc2[:])
        h2T = transpose_to(h2, BF16, id_bf, f"h2T_{t}")
        return h2, h2T, c2

    h1, h1T, c1 = lstm_step(0, h0T, c0)
    h2, h2T, c2 = lstm_step(1, h1T, c1)
    h3, h3T, c3 = lstm_step(2, h2T, c2)

    # ---- attention: all 3 attends (q = h1, h2, h3) batched in ONE pass ----
    # row index r = a*BC + b (a = attend/step, b = batch). One sweep over the
    # encoder serves all three queries: 3x less PE streaming than per-attend.
    NQ = 3 * BC  # 24
    qTs = [h1T, h2T, h3T]
    qTm = sp.tile([P, 2, BC, NQ], BF16, name="qTm", bufs=1)
    nc.vector.memset(qTm[:], 0.0)
    for ch in range(2):
        for a in range(3):
            for b in range(BC):
                nc.vector.tensor_copy(qTm[:, ch, b, a * BC + b:a * BC + b + 1],
                                      qTs[a][:, ch, b:b + 1])
    sc = sp.tile([NQ, S], F32, name="sc", bufs=1)
    for j in range(NCH):
        sps = psc.tile([NQ, 512], F32, name="sc_ps", tag="seb")
        for b in range(BC):
            for ch in range(2):
                nc.tensor.matmul(sps[:], qTm[:, ch, b, :],
                                 encT[b][:, ch, j * 512:(j + 1) * 512],
                                 start=(b == 0 and ch == 0),
                                 stop=(b == BC - 1 and ch == 1))
        nc.vector.tensor_copy(sc[:, j * 512:(j + 1) * 512], sps[:])
    mx = ep.tile([NQ, 1], F32, name="mx")
    nc.vector.reduce_max(mx[:], sc[:], axis=mybir.AxisListType.X)
    nc.vector.tensor_scalar_mul(mx[:], mx[:], -1.0)
    sm = ep.tile([NQ, 1], F32, name="sm")
    nc.scalar.activation(sc[:], sc[:], Exp, bias=mx[:], accum_out=sm[:])
    rs = ep.tile([NQ, 1], F32, name="rs")
    nc.vector.reciprocal(rs[:], sm[:])
    att = sp.tile([NQ, S], BF16, name="att", bufs=1)
    nc.vector.tensor_scalar_mul(att[:], sc[:], rs[:])
    # transpose attn to [s-partition] tiles: [128, 16, NQ]
    attT = sp.tile([P, 16, NQ], BF16, name="attT", bufs=1)
    for j in range(16):
        pt = psm.tile([P, NQ], BF16, name="pt_at", tag="ps")
        nc.tensor.transpose(pt[:], att[:, j * P:(j + 1) * P], id_bf[:NQ, :NQ])
        nc.scalar.copy(attT[:, j, :], pt[:])
    # mix: one [NQ, E] accumulation per b; rows {b, BC+b, 2*BC+b} are valid.
    # Engines can't address partition offsets, so copy the full tile,
    # PE-transpose it, and pick columns (free-dim offsets).
    mixTs = [ep.tile([P, 2, BC], BF16, name=f"mixT_t{a + 1}", bufs=2)
             for a in range(3)]
    for b in range(BC):
        mps = psm.tile([NQ, E], F32, name="mix_ps", tag="ps")
        for j in range(16):
            nc.tensor.matmul(mps[:], attT[:, j, :], encS[b][:, j, :],
                             start=(j == 0), stop=(j == 15))
        mfull = ep.tile([NQ, E], BF16, name="mfull", bufs=2)
        nc.scalar.copy(mfull[:], mps[:])
        for ch in range(2):
            pt = psm.tile([P, NQ], BF16, name="pt_mx", tag="ps")
            nc.tensor.transpose(pt[:], mfull[:, ch * P:(ch + 1) * P],
                                id_bf[:NQ, :NQ])
            for a in range(3):
                nc.vector.tensor_copy(mixTs[a][:, ch, b:b + 1],
                                      pt[:, a * BC + b:a * BC + b + 1])

    def attend_out(mixT, qT, tag):
        aps = psm.tile([BC, E], F32, name="ao_ps", tag="ps")
        for ch in range(2):
            nc.tensor.matmul(aps[:], mixT[:, ch, :], Wa_mT[:, ch, :],
                             start=(ch == 0), stop=False)
        for ch in range(2):
            nc.tensor.matmul(aps[:], qT[:, ch, :], Wa_qT[:, ch, :],
                             start=False, stop=False)
        nc.tensor.matmul(aps[:], ones_bf[:], b_attn[:], start=False, stop=True)
        o = ep.tile([BC, E], BF16, name=f"out_{tag}", bufs=1)
        nc.scalar.activation(o[:], aps[:], Tanh)
        oT = transpose_to(o, BF16, id_bf, f"outT_{tag}")
        return o, oT

    out2, out2T = attend_out(mixTs[1], h2T, "t2")
    out3, out3T = attend_out(mixTs[2], h3T, "t3")
    out1, out1T = attend_out(mixTs[0], h1T, "t1")

    # t1_out = out1 @ W_rel.T + b_rel -> out[:, 0:R]
    t1ps = psm.tile([BC, R], F32, name="t1_ps", tag="ps")
    for ch in range(2):
        nc.tensor.matmul(t1ps[:], out1T[:, ch, :], W_relT[:, ch, :],
                         start=(ch == 0), stop=False)
    nc.tensor.matmul(t1ps[:], ones_bf[:], b_rel[:], start=False, stop=True)
    t1sb = ep.tile([BC, R], F32, name="t1sb")
    nc.scalar.copy(t1sb[:], t1ps[:])
    dma(out=out_ap[:, 0:R], in_=t1sb[:])

    # ---- vbias variants: vb = o @ Kv_x + b_conv, transposed to [P,2,BC] ----
    def vbias(oT, Kv, tag):
        vps = psm.tile([BC, E], F32, name="vb_ps", tag="ps")
        for ch in range(2):
            nc.tensor.matmul(vps[:], oT[:, ch, :], Kv[:, ch, :],
                             start=(ch == 0), stop=False)
        nc.tensor.matmul(vps[:], ones_bf[:], b_conv[:], start=False, stop=True)
        vsb = ep.tile([BC, E], F32, name="vb_sb", bufs=2)
        nc.vector.tensor_copy(vsb[:], vps[:])
        return transpose_to(vsb, F32, id_f32, f"vbT_{tag}")

    vbA = [vbias(out2T, kv, f"a{i}") for i, kv in enumerate((Kv_i, Kv_f, Kv_l))]
    vbB = [vbias(out3T, kv, f"b{i}") for i, kv in enumerate((Kv_i, Kv_f, Kv_l))]

    # ---- conv + relu + entity-head reduction ----
    for b in range(BC):
        for j in range(NCH):
            s0 = j * 512
            cps = []
            for half in range(2):
                ps = pcv.tile([P, 512], F32, name="conv_ps")
                first = True
                # center tap (w=1) first: always full width, so the start=True
                # matmul initializes every psum element before partial taps add
                for w in (1, 0, 2):
                    lo = s0 + w - 1
                    ob, oe = 0, 512
                    if lo < 0:
                        ob, lo = 1, 0
                    elif lo + 512 > S:
                        oe = 511
                    for ch in range(2):
                        nc.tensor.matmul(ps[:, ob:oe], Kenc[:, w, ch, half, :],
                                         encT[b][:, ch, lo:lo + (oe - ob)],
                                         start=first, stop=(w == 2 and ch == 1))
                        first = False
                cps.append(ps)
            for v, vbs in enumerate((vbA, vbB)):
                ent_ps = psc.tile([2, 512], F32, name="ent_ps", tag="seb")
                for half in range(2):
                    r = rp.tile([P, 512], BF16, name="relu")
                    if v == 0:
                        nc.scalar.activation(r[:], cps[half][:], Relu,
                                             bias=vbs[0][:, half, b:b + 1])
                        if j == 0:
                            nc.scalar.activation(r[:, 0:1], cps[half][:, 0:1], Relu,
                                                 bias=vbs[1][:, half, b:b + 1])
                        if j == NCH - 1:
                            nc.scalar.activation(r[:, 511:512], cps[half][:, 511:512],
                                                 Relu, bias=vbs[2][:, half, b:b + 1])
                    else:
                        nc.vector.tensor_scalar(r[:], cps[half][:],
                                                vbs[0][:, half, b:b + 1], 0.0,
                                                op0=mybir.AluOpType.add,
                                                op1=mybir.AluOpType.max)
                        if j == 0:
                            nc.vector.tensor_scalar(r[:, 0:1], cps[half][:, 0:1],
                                                    vbs[1][:, half, b:b + 1], 0.0,
                                                    op0=mybir.AluOpType.add,
                                                    op1=mybir.AluOpType.max)
                        if j == NCH - 1:
                            nc.vector.tensor_scalar(r[:, 511:512], cps[half][:, 511:512],
                                                    vbs[2][:, half, b:b + 1], 0.0,
                                                    op0=mybir.AluOpType.add,
                                                    op1=mybir.AluOpType.max)
                    nc.tensor.matmul(ent_ps[:], Went[:, half, :], r[:],
                                     start=(half == 0), stop=(half == 1))
                esb = ep.tile([2, 512], F32, name="esb", bufs=6)
                if (b + j + v) % 2 == 0:
                    nc.vector.tensor_scalar_add(esb[:], ent_ps[:], bent[:])
                else:
                    nc.scalar.activation(esb[:], ent_ps[:],
                                         mybir.ActivationFunctionType.Identity,
                                         bias=bent[:])
                dma(out=out_ap[b:b + 1, OC[2 * v] + s0:OC[2 * v] + s0 + 512],
                    in_=esb[0:1, :])
                dma(out=out_ap[b:b + 1, OC[2 * v + 1] + s0:OC[2 * v + 1] + s0 + 512],
                    in_=esb[1:2, :])


def build_nc():
    nc = bacc.Bacc("TRN2", target_bir_lowering=False, debug=False)
    io = {}

    def din(name, shape, dt):
        io[name] = nc.dram_tensor(name, shape, dt, kind="ExternalInput")

    din("enc_cs", [BC, E, S], BF16)
    din("enc_sc", [BC, S, E], BF16)
    din("W_ihT", [128, 2, 4 * E], BF16)
    din("W_hhT", [128, 2, 4 * E], BF16)
    din("Wa_mT", [128, 2, E], BF16)
    din("Wa_qT", [128, 2, E], BF16)
    din("Kv_i", [128, 2, E], BF16)
    din("Kv_f", [128, 2, E], BF16)
    din("Kv_l", [128, 2, E], BF16)
    din("Kenc", [128, 3, 2, 2, 128], BF16)
    din("W_relT", [128, 2, R], BF16)
    din("Went", [128, 2, 2], BF16)
    din("bias_g", [1, 4 * E], BF16)
    din("b_attn", [1, E], BF16)
    din("b_conv", [1, E], BF16)
    din("b_rel", [1, R], BF16)
    din("bent", [2, 1], F32)
    din("xT", [128, 3, 2, BC], BF16)
    din("h0T", [128, 2, BC], BF16)
    din("c0", [BC, E], F32)
    io["out"] = nc.dram_tensor("out", [BC, R + 4 * S], F32, kind="ExternalOutput")

    with ExitStack() as ctx:
        t = ctx.enter_context(tile.TileContext(nc))
        _emit(ctx, t, nc, io)
    nc.compile()
    return nc


def _pack2(w):  # [256, N] fp32 -> [128, 2, N]
    return np.ascontiguousarray(w.reshape(2, 128, -1).transpose(1, 0, 2))


def prepare_in_maps(inputs):
    bf = ml_dtypes.bfloat16
    enc = np.asarray(inputs["encoder_o"], np.float32)
    enc_bf = enc.astype(bf)
    enc_cs = np.ascontiguousarray(enc_bf.transpose(0, 2, 1))
    W_ih = np.asarray(inputs["W_ih"], np.float32)
    W_hh = np.asarray(inputs["W_hh"], np.float32)
    W_attn = np.asarray(inputs["W_attn"], np.float32)
    kern = np.asarray(inputs["W_conv"], np.float32).transpose(2, 1, 0)  # [3,2E,E]
    Kenc_ = kern[:, :E, :]
    Kv = kern[:, E:, :]
    Kv_i, Kv_f, Kv_l = Kv.sum(0), Kv[1] + Kv[2], Kv[0] + Kv[1]
    # Kenc pack [128, 3, 2, 2, 128]: [p,w,ch,half,m] = Kenc_[w, ch*128+p, half*128+m]
    kp = Kenc_.reshape(3, 2, 128, 2, 128).transpose(2, 0, 1, 3, 4)
    We = np.stack([np.asarray(inputs["W_ent1"])[0], np.asarray(inputs["W_ent2"])[0]], 1)
    x1 = np.broadcast_to(np.asarray(inputs["sos_emb"])[0], (B, E))
    x2 = np.asarray(inputs["rel_emb"])[np.asarray(inputs["r_in"]).astype(np.int64)]
    idx = np.arange(B)
    k1 = np.asarray(inputs["k1"])[:, 0].astype(np.int64)
    k2 = np.asarray(inputs["k2"])[:, 0].astype(np.int64)
    x3 = enc[idx, k1] + enc[idx, k2]
    X = np.stack([x1, x2, x3], 0).astype(np.float32)      # [3,B,E]
    h0 = np.asarray(inputs["h0"], np.float32)[0]
    c0 = np.asarray(inputs["c0"], np.float32)

    shared = {
        "W_ihT": _pack2(W_ih.T).astype(bf),
        "W_hhT": _pack2(W_hh.T).astype(bf),
        "Wa_mT": _pack2(W_attn[:, :E].T).astype(bf),
        "Wa_qT": _pack2(W_attn[:, E:].T).astype(bf),
        "Kv_i": _pack2(Kv_i).astype(bf),
        "Kv_f": _pack2(Kv_f).astype(bf),
        "Kv_l": _pack2(Kv_l).astype(bf),
        "Kenc": np.ascontiguousarray(kp).astype(bf),
        "W_relT": _pack2(np.asarray(inputs["W_rel"], np.float32).T).astype(bf),
        "Went": _pack2(We).astype(bf),
        "bias_g": (np.asarray(inputs["b_ih"], np.float32)
                   + np.asarray(inputs["b_hh"], np.float32))[None].astype(bf),
        "b_attn": np.asarray(inputs["b_attn"], np.float32)[None].astype(bf),
        "b_conv": np.asarray(inputs["b_conv"], np.float32)[None].astype(bf),
        "b_rel": np.asarray(inputs["b_rel"], np.float32)[None].astype(bf),
        "bent": np.array([[np.asarray(inputs["b_ent1"]).ravel()[0]],
                          [np.asarray(inputs["b_ent2"]).ravel()[0]]], np.float32),
    }
    in_maps = []
    for c in range(NCORES):
        sl = slice(c * BC, (c + 1) * BC)
        m = dict(shared)
        m["enc_cs"] = np.ascontiguousarray(enc_cs[sl])
        m["enc_sc"] = np.ascontiguousarray(enc_bf[sl])
        # xT pack [128, 3, 2, BC]
        xs = X[:, sl]                                      # [3,BC,E]
        m["xT"] = np.ascontiguousarray(
            xs.transpose(2, 0, 1).reshape(2, 128, 3, BC).transpose(1, 2, 0, 3)
        ).astype(bf)
        m["h0T"] = np.ascontiguousarray(
            h0[sl].T.reshape(2, 128, BC).transpose(1, 0, 2)).astype(bf)
        m["c0"] = np.ascontiguousarray(c0[0, sl]) if c0.ndim == 3 else \
            np.ascontiguousarray(c0[sl])
        in_maps.append(m)
    return in_maps


_NC_CACHE = {}


def get_nc():
    if "nc" not in _NC_CACHE:
        _NC_CACHE["nc"] = build_nc()
    return _NC_CACHE["nc"]


def kernel(**inputs) -> np.ndarray:
    nc = get_nc()
    in_maps = prepare_in_maps(inputs)
    res = run_bass_kernel_spmd(nc, in_maps, core_ids=list(range(NCORES)))
    return np.concatenate([r["out"] for r in res.results], 0).astype(np.float32)


if __name__ == "__main__":
    import jax
    import reference as refmod
    with jax.default_device(jax.devices("cpu")[0]):
        inputs = {k: np.asarray(v) for k, v in refmod.setup_inputs().items()}
        expected = np.asarray(refmod.reference(**inputs))
    actual = kernel(**inputs)
    err = np.abs(actual - expected)
    print("max abs err:", err.max(), "rel:", err.max() / np.abs(expected).max())
